# revision 30
# baseline (speedup 1.0000x reference)
"""Trainium2 Bass kernel for nn_EpisodicMemory (BitNet projections + memory cross-attention).

kernel(**inputs) takes FULL unsharded numpy inputs, returns FULL output
[8, 4096, 1024] f32. Batch-parallel across 8 NeuronCores; two scalar
AllReduce(max) collectives provide the global BitNet activation scales.

Design — algebraic fusion of the first BitNet matmul into the memory bank:
  sims = qk @ mk^T,  qk = (qx @ qWk^T)*sck + bk
       = sck * qx @ (qWk^T @ mk^T) + bk @ mk^T
  FT[i, m] := sum_o qWk[o, i] * mk[m, o]   (precomputed once on PE, 2.1 GMAC)
  bmk[m]   := sum_o bk[o] * mk[m, o]       (folded into the exp bias)
This removes the per-tile qk stage (4.3 GMAC + an ACT pass) and all Wk
transposes (qWk is consumed in natural [o, i] layout as the FT lhsT).

Math per core (batch element), matching the reference:
  s_x   = max|x| (global) / 127 ;  qx = rne(x/s_x)  (ints, exact in fp16)
  qWk   = sign(Wk) * (|Wk| > 0.5*mean|Wk|)          (ternary, exact in fp16)
  probs = softmax((sck * qx@FT + bmk) / sqrt(Dm))   (exp with -8 logit shift)
  h     = x + probs @ mv
  out   = rne(h/s_h) @ qWo^T * (mean|Wo| * s_h) + bo,  s_h = max|h|(global)/127
All matmuls fp16 on PE (BitNet ones exact); softmax/quantization f32.

Engine-FIFO discipline (every engine queue is in-order, so an op whose
dependency lands late blocks everything emitted after it on that engine):
  - sync ring   : x loads + all latency-critical transposed reads (qxT, hT)
  - scalar ring : stores (qx_b, h_sp, wo_b) + the off-critical qwoT reads
  - gpsimd ring : weight/bias load issues (dep-free) + x-max reduces + out
                  stores; nothing here may depend on a late producer
  - phase-gate scalar chains (s_x/s_h partition reduce + broadcasts, weight
    means, ternary thresholds) run as PE transpose-reduce + ones-matmul
    broadcasts so they never sit in the Pool FIFO
  - Wo prep: mean pass at B tile 0, quantize spread over tiles 1-4 (its DVE
    ops sit in the FIFO exactly where their data lands), transposes via the
    scalar ring at tiles 3-4; all consumed only in phase C
h is bounced through DRAM once (fp16 natural write, transposed read in
phase C; 2 slots + per-ib-block requantize keep the C pipeline fed).
"""

import math
import time

import numpy as np

import concourse.bass as bass
import concourse.tile as tile
from concourse import bacc, bass_isa, mybir
from concourse.bass_utils import run_bass_kernel_spmd

F32 = mybir.dt.float32
F16 = mybir.dt.float16

N_CORES = 8
MAGIC = 1.5 * (2.0 ** 23)   # fp32 RNE rounding trick
EXP_SHIFT = 8.0             # logit shift so exp() fits fp16

B, S_FULL, E_DIM, DM_DIM, M_DIM = 8, 4096, 1024, 1024, 2048


def build_nc(S=S_FULL, E=E_DIM, DM=DM_DIM, M=M_DIM, T=512, repeat=1,
             use_collectives=True, with_bias=True, with_bk=False):
    assert S % T == 0 and T % 128 == 0 and E % 512 == 0 and DM % 128 == 0 and M % 128 == 0
    NT = S // T           # token tiles
    TS = T // 128         # 128-token sub-blocks per tile
    NE = E // 128         # embed 128-blocks
    ND = DM // 128        # memory-dim 128-blocks
    NM = M // 128         # memory-size 128-blocks
    NOH = E // 512        # output 512-chunks
    NMC = M // 512        # memory 512-chunks
    NCH = E // 512        # embed 512-chunks

    nc = bacc.Bacc("TRN2", target_bir_lowering=False, debug=False,
                   num_devices=N_CORES)

    x_in = nc.dram_tensor("x", [S, E], F32, kind="ExternalInput").ap()
    mk_in = nc.dram_tensor("memory_keys", [M, DM], F32, kind="ExternalInput").ap()
    mv_in = nc.dram_tensor("memory_values", [M, E], F32, kind="ExternalInput").ap()
    wk_in = nc.dram_tensor("Wk", [DM, E], F32, kind="ExternalInput").ap()
    bk_in = nc.dram_tensor("bk", [DM], F32, kind="ExternalInput").ap()
    wo_in = nc.dram_tensor("Wo", [E, E], F32, kind="ExternalInput").ap()
    bo_in = nc.dram_tensor("bo", [E], F32, kind="ExternalInput").ap()
    out_ext = nc.dram_tensor("out", [S, E], F32, kind="ExternalOutput").ap()

    import contextlib
    with tile.TileContext(nc) as tc:
        loop_cm = tc.For_i(0, repeat, 1) if repeat > 1 else contextlib.nullcontext()
        with loop_cm:
          with (
            tc.tile_pool(name="pp", bufs=1) as pp,
            tc.tile_pool(name="wp", bufs=2) as wp,
            tc.tile_pool(name="psp", bufs=2, space="PSUM") as psp,
            tc.tile_pool(name="dp", bufs=1, space="DRAM") as dp,
          ):
            # ---------------- persistent SBUF ----------------
            ftm = pp.tile([128, NE, M], F16, tag="ftm")        # FT[i, m]
            mv_sb = pp.tile([128, NM, E], F16, tag="mv_sb")    # mv[m, e]
            qwoT = pp.tile([128, NE, E], F16, tag="qwoT")      # qWo^T[i, o]
            if with_bias:
                bo_sb = pp.tile([1, E], F32, tag="bo_sb")
                bo_sc = pp.tile([1, E], F16, tag="bo_sc")
                ones_row = pp.tile([1, 128], F16, tag="ones_row")
                nc.vector.memset(ones_row[:], 1.0)
            ones_col = pp.tile([128, 8], F16, tag="ones_col")
            ones_bc = pp.tile([1, 128], F32, tag="ones_bc")
            hmax_buf = pp.tile([128, NT * TS * NOH], F32, tag="hmax_buf")
            xmax_buf = pp.tile([128, NT], F32, tag="xmax_buf")
            xmax_red = pp.tile([128, 1], F32, tag="xmax_red")
            hmax_red = pp.tile([128, 1], F32, tag="hmax_red")

            sc = {}
            for nm in ("gmax_x", "s_x", "inv_sx", "ws_k", "thr_k", "nthr_k",
                       "ws_o", "thr_o", "nthr_o", "sck32", "gmax_h", "s_h",
                       "inv_sh", "sco", "inv_sco", "red1", "red2"):
                sc[nm] = pp.tile([1, 8], F32, name=f"sc_{nm}", tag=f"sc_{nm}")
            bc = {}
            for nm in ("inv_sx", "thr_k", "nthr_k", "thr_o", "nthr_o", "sck32",
                       "inv_sh", "sco"):
                bc[nm] = pp.tile([128, 1], F32, name=f"bc_{nm}", tag=f"bc_{nm}")

            neg_shift = pp.tile([128, 1], F32, tag="neg_shift")
            ident32 = pp.tile([128, 128], F32, tag="ident32")
            nc.vector.memset(ones_col[:], 1.0)
            nc.vector.memset(ones_bc[:], 1.0)
            nc.vector.memset(neg_shift[:], -EXP_SHIFT)
            from concourse.masks import make_identity
            make_identity(nc, ident32[:])

            # exp bias: -shift, plus bmk[m]/sqrt(Dm) when bk != 0
            if with_bk:
                ebias = pp.tile([128, NM], F32, tag="ebias")
                bk_sb = pp.tile([128, ND], F32, tag="bk_sb")

            # ---------------- DRAM scratch ----------------
            h_sp = [dp.tile([T, E], F16, tag="h_spill", bufs=NT, name=f"hsp{i}")
                    for i in range(NT)]

            rg = [list(range(N_CORES))]

            def allreduce_max(src_scalar, dst_scalar):
                if not use_collectives:
                    nc.vector.tensor_copy(dst_scalar[0:1, 0:1], src_scalar[0:1, 0:1])
                    return
                ccin = dp.tile([1, 8], F32, tag="cc_in", bufs=2, name="ccin")
                ccout = dp.tile([1, 8], F32, addr_space="Shared", tag="cc_out",
                                bufs=2, name="ccout")
                nc.sync.dma_start(ccin[:], src_scalar[:])
                nc.gpsimd.collective_compute(
                    "AllReduce", mybir.AluOpType.max, replica_groups=rg,
                    ins=[ccin[:]], outs=[ccout[:]])
                nc.sync.dma_start(dst_scalar[:], ccout[:])

            def part_reduce_pe(vec128, out_scalar, op):
                # cross-partition reduce without touching the Pool FIFO:
                # PE transpose to one partition, then a DVE free-dim reduce
                tp = psp.tile([128, 128], F32, tag="ps_d", name="prp_ps")
                nc.tensor.transpose(tp[0:1, 0:128], vec128[:, 0:1], ident32[:])
                nc.vector.tensor_reduce(
                    out_scalar[0:1, 0:1], tp[0:1, 0:128],
                    axis=mybir.AxisListType.X,
                    op=(mybir.AluOpType.max if op == "max" else mybir.AluOpType.add))

            def bcast_pe(src_scalar, bc_tile):
                # scalar -> all partitions via rank-1 ones matmul (PE + DVE)
                bp = psp.tile([128, 8], F32, tag="ps_d", name="bc_ps")
                nc.tensor.matmul(bp[:, 0:1], ones_bc[0:1, :],
                                 src_scalar[0:1, 0:1], start=True, stop=True)
                nc.vector.tensor_copy(bc_tile[:], bp[:, 0:1])

            # ============ PHASE A ============
            # x streams on the sync ring with abs-max reduces on gpsimd; the
            # weight loads (gpsimd ring issues, dep-free), ternary prep (DVE),
            # memory-bank transposes + FT build (PE) all run concurrently.
            with tc.tile_pool(name="prep", bufs=1) as prep:
                # --- Wk: abs-mean pass (streamed loads + DVE reduces) ---
                wacc = wp.tile([128, ND * NCH], F32, tag="wacc", bufs=1, name="wacc")
                for ob in range(ND):
                    for ch in range(NCH):
                        cf = slice(ch * 512, (ch + 1) * 512)
                        wt = wp.tile([128, 512], F32, tag="wld", bufs=3, name="wtk")
                        nc.gpsimd.dma_start(wt[:], wk_in[ob * 128:(ob + 1) * 128, cf])
                        nc.vector.tensor_reduce(
                            wacc[:, ob * NCH + ch:ob * NCH + ch + 1], wt[:],
                            axis=mybir.AxisListType.X, op=mybir.AluOpType.add,
                            apply_absolute_value=True)

                # --- mk half 0: PE transposes (4 share one PSUM bank -> one
                #     wide DVE copy); runs while the Wk mean pass streams ---
                NMH = NM // 2

                def load_mkT_half(half):
                    mkT = prep.tile([128, ND, M // 2], F16, tag="mkT", name="mkT")
                    for mb in range(NMH):
                        mrow = (half * NMH + mb) * 128
                        for ch in range(DM // 512):
                            mkt = wp.tile([128, 512], F32, tag="wld", bufs=3,
                                          name="mkt")
                            nc.gpsimd.dma_start(
                                mkt[:], mk_in[mrow:mrow + 128,
                                              ch * 512:(ch + 1) * 512])
                            tps = psp.tile([128, 512], F32, tag="ps_d",
                                           name="mk_ps")
                            for j in range(4):
                                nc.tensor.transpose(
                                    tps[:, j * 128:(j + 1) * 128],
                                    mkt[:, j * 128:(j + 1) * 128], ident32[:])
                            nc.vector.tensor_copy(
                                mkT[:, ch * 4:(ch + 1) * 4, mb * 128:(mb + 1) * 128],
                                tps[:].rearrange("p (a b) -> p a b", a=4))
                    return mkT

                mkT0 = load_mkT_half(0)

                # --- Wk thresholds (PE-path reduce/broadcast) + quantize pass
                #     (into natural [o, i] fp16 — the FT lhsT, no transpose) ---
                waccr = wp.tile([128, 1], F32, tag="waccr", name="waccr")
                nc.vector.tensor_reduce(waccr[:], wacc[:], axis=mybir.AxisListType.X,
                                        op=mybir.AluOpType.add)
                part_reduce_pe(waccr, sc["red2"], "add")
                nc.vector.tensor_scalar(sc["ws_k"][0:1, 0:1], sc["red2"][0:1, 0:1],
                                        1.0 / (DM * E), None,
                                        op0=mybir.AluOpType.mult)
                nc.vector.tensor_scalar(sc["thr_k"][0:1, 0:1], sc["ws_k"][0:1, 0:1],
                                        0.5, None, op0=mybir.AluOpType.mult)
                nc.vector.tensor_scalar(sc["nthr_k"][0:1, 0:1], sc["thr_k"][0:1, 0:1],
                                        -1.0, None, op0=mybir.AluOpType.mult)
                bcast_pe(sc["thr_k"], bc["thr_k"])
                bcast_pe(sc["nthr_k"], bc["nthr_k"])
                qwk = prep.tile([128, ND, E], F16, tag="qwk")
                for ob in range(ND):
                    for ch in range(NCH):
                        cf = slice(ch * 512, (ch + 1) * 512)
                        wt = wp.tile([128, 512], F32, tag="wld", bufs=3, name="wtk2")
                        nc.gpsimd.dma_start(wt[:], wk_in[ob * 128:(ob + 1) * 128, cf])
                        ge = wp.tile([128, 512], F32, tag="f32b", bufs=3, name="ge")
                        nc.vector.tensor_scalar(ge[:], wt[:],
                                                bc["thr_k"][:, 0:1], None,
                                                op0=mybir.AluOpType.is_gt)
                        le = wp.tile([128, 512], F32, tag="f32b", bufs=3, name="le")
                        nc.vector.tensor_scalar(le[:], wt[:],
                                                bc["nthr_k"][:, 0:1], None,
                                                op0=mybir.AluOpType.is_lt)
                        nc.vector.tensor_tensor(qwk[:, ob, cf], ge[:], le[:],
                                                op=mybir.AluOpType.subtract)

                # --- FT halves + bmk; half 1's mk loads prefetch under the
                #     half 0 matmuls ---
                if with_bk:
                    nc.gpsimd.dma_start(bk_sb[:],
                                        bk_in.rearrange("(b p) -> p b", p=128))
                    bmk = wp.tile([128, NM], F32, tag="bmk", bufs=1, name="bmk")

                def ft_half(half, mkT):
                    for ib in range(NE):
                        for mc in range(NMC // 2):
                            mf = slice(mc * 512, (mc + 1) * 512)
                            gf = slice(half * (M // 2) + mc * 512,
                                       half * (M // 2) + (mc + 1) * 512)
                            ps = psp.tile([128, 512], F32, tag="ps_a", name="ft_ps")
                            for ob in range(ND):
                                nc.tensor.matmul(
                                    ps[:], qwk[:, ob, ib * 128:(ib + 1) * 128],
                                    mkT[:, ob, mf], start=(ob == 0),
                                    stop=(ob == ND - 1))
                            nc.scalar.activation(ftm[:, ib, gf], ps[:],
                                                 mybir.ActivationFunctionType.Copy)
                    if with_bk:
                        bmk_ps = psp.tile([128, 8], F32, tag="ps_d", name="bmk_ps")
                        for mb in range(NMH):
                            for ob in range(ND):
                                nc.tensor.matmul(
                                    bmk_ps[:, 0:1],
                                    mkT[:, ob, mb * 128:(mb + 1) * 128],
                                    bk_sb[:, ob:ob + 1],
                                    start=(ob == 0), stop=(ob == ND - 1))
                            nc.vector.tensor_copy(bmk[:, half * NMH + mb:
                                                      half * NMH + mb + 1],
                                                  bmk_ps[:, 0:1])

                ft_half(0, mkT0)
                mkT1 = load_mkT_half(1)
                ft_half(1, mkT1)
                if with_bk:
                    nc.vector.tensor_scalar(
                        ebias[:], bmk[:], 1.0 / math.sqrt(DM), -EXP_SHIFT,
                        op0=mybir.AluOpType.mult, op1=mybir.AluOpType.add)

                # --- memory values: f32 -> f16 cast on ACT (DVE is busy) ---
                for mb in range(NM):
                    for ch in range(E // 512):
                        mvt = wp.tile([128, 512], F32, tag="wld", bufs=3, name="mvt")
                        nc.gpsimd.dma_start(mvt[:], mv_in[mb * 128:(mb + 1) * 128,
                                                          ch * 512:(ch + 1) * 512])
                        nc.scalar.activation(mv_sb[:, mb, ch * 512:(ch + 1) * 512],
                                             mvt[:],
                                             mybir.ActivationFunctionType.Copy)

                # --- x abs-max pass: loads on sync, reduces on gpsimd so the
                #     DVE FIFO stays clear for the weight prep ---
                chunk_order = list(range(2, NT)) + [0, 1]
                xa_ref = {}
                xmax_fine = pp.tile([128, 2 * TS], F32, tag="xmax_fine")
                for c in chunk_order:
                    xc = wp.tile([128, TS, E], F32, tag="x_nat", name="xc")
                    if c in (0, 1):
                        # quarter-granularity so the final reduce (gating
                        # AllReduce #1) is short
                        for n in range(TS):
                            nc.sync.dma_start(
                                xc[:, n, :],
                                x_in[c * T:(c + 1) * T, :]
                                .rearrange("(n p) d -> p n d", p=128)[:, n, :])
                            nc.vector.tensor_reduce(
                                xmax_fine[:, c * TS + n:c * TS + n + 1], xc[:, n, :],
                                axis=mybir.AxisListType.X,
                                op=mybir.AluOpType.max, apply_absolute_value=True)
                        nc.vector.tensor_reduce(
                            xmax_buf[:, c:c + 1], xmax_fine[:, c * TS:(c + 1) * TS],
                            axis=mybir.AxisListType.X, op=mybir.AluOpType.max)
                    else:
                        nc.sync.dma_start(
                            xc[:], x_in[c * T:(c + 1) * T, :]
                            .rearrange("(n p) d -> p n d", p=128))
                        nc.vector.tensor_reduce(
                            xmax_buf[:, c:c + 1], xc[:], axis=mybir.AxisListType.XY,
                            op=mybir.AluOpType.max, apply_absolute_value=True)
                    xa_ref[c] = xc
                nc.vector.tensor_reduce(xmax_red[:], xmax_buf[:],
                                        axis=mybir.AxisListType.X,
                                        op=mybir.AluOpType.max)
                part_reduce_pe(xmax_red, sc["red1"], "max")
                allreduce_max(sc["red1"], sc["gmax_x"])
                nc.vector.tensor_scalar(sc["s_x"][0:1, 0:1], sc["gmax_x"][0:1, 0:1],
                                        1.0 / 127.0, None, op0=mybir.AluOpType.mult)
                nc.vector.reciprocal(sc["inv_sx"][0:1, 0:1], sc["s_x"][0:1, 0:1])
                bcast_pe(sc["inv_sx"], bc["inv_sx"])

                # exp scale: sck / sqrt(Dm) = ws_k * s_x / 32
                nc.vector.tensor_tensor(sc["sck32"][0:1, 0:1], sc["ws_k"][0:1, 0:1],
                                        sc["s_x"][0:1, 0:1], op=mybir.AluOpType.mult)
                nc.vector.tensor_scalar(sc["sck32"][0:1, 0:1], sc["sck32"][0:1, 0:1],
                                        1.0 / math.sqrt(DM), None,
                                        op0=mybir.AluOpType.mult)
                bcast_pe(sc["sck32"], bc["sck32"])


            # ======================= PHASE B =======================
            def load_x(it):
                if it in (0, 1):
                    return xa_ref[it]
                x_nat = wp.tile([128, TS, E], F32, tag="x_nat", name="x_nat")
                nc.sync.dma_start(
                    x_nat[:], x_in[it * T:(it + 1) * T, :]
                    .rearrange("(n p) d -> p n d", p=128))
                return x_nat

            def prep_b(it, x_nat=None):
                """Quantize + bounce + transpose for tile `it`.
                Returns (x_nat, qxT)."""
                if x_nat is None:
                    x_nat = load_x(it)
                qx_b = dp.tile([T, E], F16, tag="qx_b", bufs=2, name="qx_b")
                if not with_bias:
                    qx_nat = wp.tile([128, TS, E], F16, tag="qx_nat", bufs=1,
                                     name="qx_nat")
                for n in range(TS):
                    for ch in range(NCH):
                        off = ch * 512
                        t1 = wp.tile([128, 512], F32, tag="f32b", bufs=3, name="t1")
                        nc.vector.tensor_scalar(
                            t1[:], x_nat[:, n, off:off + 512], bc["inv_sx"][:, 0:1],
                            MAGIC, op0=mybir.AluOpType.mult, op1=mybir.AluOpType.add)
                        if with_bias:
                            # bias build trades the big staging buffer for
                            # chunked stores (SBUF headroom)
                            qch = wp.tile([128, 512], F16, tag="h16", bufs=3,
                                          name="qch")
                            nc.vector.tensor_scalar(
                                qch[:], t1[:], MAGIC, None,
                                op0=mybir.AluOpType.subtract)
                            nc.scalar.dma_start(
                                qx_b[:].rearrange("(n p) d -> p n d", p=128)
                                [:, n, off:off + 512], qch[:])
                        else:
                            nc.vector.tensor_scalar(
                                qx_nat[:, n, off:off + 512], t1[:], MAGIC, None,
                                op0=mybir.AluOpType.subtract)
                if not with_bias:
                    nc.scalar.dma_start(
                        qx_b[:].rearrange("(n p) d -> p n d", p=128), qx_nat[:])
                qxT = wp.tile([128, NE, T], F16, tag="qxT", bufs=2, name="qxT")
                for ib in range(NE):
                    nc.sync.dma_start_transpose(qxT[:, ib, :],
                                                qx_b[:, ib * 128:(ib + 1) * 128])
                return x_nat, qxT

            def wo_prep_step(it):
                # Wo mean pass at tile 0; quantize 2 o-blocks per tile over
                # tiles 1-4; qwoT transposed reads (scalar ring) tiles 3-4.
                # Loads ride the gpsimd ring (idle during phase B).
                if it == 0:
                    wacc_o = wp.tile([128, NE * NCH], F32, tag="wacc", bufs=1,
                                     name="wacc_o")
                    for ob in range(NE):
                        for ch in range(NCH):
                            cf = slice(ch * 512, (ch + 1) * 512)
                            wt = wp.tile([128, 512], F32, tag="wld", bufs=3,
                                         name="wt")
                            nc.gpsimd.dma_start(
                                wt[:], wo_in[ob * 128:(ob + 1) * 128, cf])
                            nc.vector.tensor_reduce(
                                wacc_o[:, ob * NCH + ch:ob * NCH + ch + 1], wt[:],
                                axis=mybir.AxisListType.X, op=mybir.AluOpType.add,
                                apply_absolute_value=True)
                    waccr_o = wp.tile([128, 1], F32, tag="waccr", name="waccr_o")
                    nc.vector.tensor_reduce(waccr_o[:], wacc_o[:],
                                            axis=mybir.AxisListType.X,
                                            op=mybir.AluOpType.add)
                    part_reduce_pe(waccr_o, sc["red2"], "add")
                    nc.vector.tensor_scalar(sc["ws_o"][0:1, 0:1],
                                            sc["red2"][0:1, 0:1],
                                            1.0 / (E * E), None,
                                            op0=mybir.AluOpType.mult)
                    nc.vector.tensor_scalar(sc["thr_o"][0:1, 0:1],
                                            sc["ws_o"][0:1, 0:1],
                                            0.5, None, op0=mybir.AluOpType.mult)
                    nc.vector.tensor_scalar(sc["nthr_o"][0:1, 0:1],
                                            sc["thr_o"][0:1, 0:1], -1.0, None,
                                            op0=mybir.AluOpType.mult)
                    bcast_pe(sc["thr_o"], bc["thr_o"])
                    bcast_pe(sc["nthr_o"], bc["nthr_o"])
                    if with_bias:
                        nc.gpsimd.dma_start(bo_sb[0:1, :],
                                            bo_in.rearrange("(a e) -> a e", a=1))
                elif it <= 4:
                    part = it - 1
                    for ob in range(part * 2, part * 2 + 2):
                        for ch in range(NCH):
                            cf = slice(ch * 512, (ch + 1) * 512)
                            wt = wp.tile([128, 512], F32, tag="wld", bufs=3,
                                         name="wt2")
                            nc.gpsimd.dma_start(
                                wt[:], wo_in[ob * 128:(ob + 1) * 128, cf])
                            ge = wp.tile([128, 512], F32, tag="f32b", bufs=3,
                                         name="geo")
                            nc.vector.tensor_scalar(ge[:], wt[:],
                                                    bc["thr_o"][:, 0:1],
                                                    None, op0=mybir.AluOpType.is_gt)
                            le = wp.tile([128, 512], F32, tag="f32b", bufs=3,
                                         name="leo")
                            nc.vector.tensor_scalar(le[:], wt[:],
                                                    bc["nthr_o"][:, 0:1],
                                                    None, op0=mybir.AluOpType.is_lt)
                            q16 = wp.tile([128, 512], F16, tag="w16", name="q16")
                            nc.vector.tensor_tensor(q16[:], ge[:], le[:],
                                                    op=mybir.AluOpType.subtract)
                            nc.scalar.dma_start(
                                wo_b[ob * 128:(ob + 1) * 128, cf], q16[:])
                    if it in (3, 4):
                        # transposed column-stripe reads need all o-blocks of
                        # the stripe; rows ob 0..7 are complete by tile 3 for
                        # stripes 0-3 only after tile 4 for 4-7 -> split 4+4
                        for ib in range((it - 3) * 4, (it - 3) * 4 + 4):
                            nc.scalar.dma_start_transpose(
                                qwoT[:, ib, :], wo_b[0:E, ib * 128:(ib + 1) * 128])

            wo_b = dp.tile([E, E], F16, tag="w_b", bufs=2, name="wo_b")
            nxt = prep_b(0)
            for it in range(NT):
                x_nat, qxT = nxt
                # issue next tile's x load first: a full sims window of runway
                x_next = load_x(it + 1) if it + 1 < NT else None

                # simsT -> exp (shifted) -> fp16, [M partitions, T free]
                expT = wp.tile([128, NM, T], F16, tag="expT", bufs=1, name="expT")
                for mb in range(NM):
                    for tch in range(T // 512):
                        tf = slice(tch * 512, (tch + 1) * 512)
                        ps = psp.tile([128, 512], F32, tag="ps_b", name="sims_ps")
                        for ib in range(NE):
                            nc.tensor.matmul(
                                ps[:], ftm[:, ib, mb * 128:(mb + 1) * 128],
                                qxT[:, ib, tf], start=(ib == 0), stop=(ib == NE - 1))
                        if with_bk:
                            nc.scalar.activation(
                                expT[:, mb, tf], ps[:],
                                mybir.ActivationFunctionType.Exp,
                                bias=ebias[:, mb:mb + 1], scale=bc["sck32"][:, 0:1])
                        else:
                            nc.scalar.activation(
                                expT[:, mb, tf], ps[:],
                                mybir.ActivationFunctionType.Exp,
                                bias=neg_shift[:, 0:1], scale=bc["sck32"][:, 0:1])

                # software pipeline: emit next tile's prep early (after sims)
                if it + 1 < NT:
                    nxt = prep_b(it + 1, x_next)

                # retrieved (natural layout) + softmax denominator
                for tsub in range(TS):
                    tcol = slice(tsub * 128, (tsub + 1) * 128)
                    hidx = (it * TS + tsub) * NOH
                    dps = psp.tile([128, 8], F32, tag="ps_d", name="den_ps")
                    inv_t = wp.tile([128, 1], F32, tag="inv_t", bufs=4, name="inv_t")
                    for eh in range(NOH):
                        ef = slice(eh * 512, (eh + 1) * 512)
                        rps = psp.tile([128, 512], F32, tag="ps_r", name="r_ps")
                        for mb in range(NM):
                            nc.tensor.matmul(rps[:], expT[:, mb, tcol],
                                             mv_sb[:, mb, ef],
                                             start=(mb == 0), stop=(mb == NM - 1))
                            if eh == 0:
                                nc.tensor.matmul(dps[:, 0:1], expT[:, mb, tcol],
                                                 ones_col[:, 0:1],
                                                 start=(mb == 0), stop=(mb == NM - 1))
                        if eh == 0:
                            nc.vector.reciprocal(inv_t[:], dps[:, 0:1])
                        hch = wp.tile([128, 512], F16, tag="h16", bufs=3, name="hch")
                        nc.vector.scalar_tensor_tensor(
                            hch[:], rps[:], inv_t[:, 0:1], x_nat[:, tsub, ef],
                            op0=mybir.AluOpType.mult, op1=mybir.AluOpType.add)
                        nc.vector.tensor_reduce(
                            hmax_buf[:, hidx + eh:hidx + eh + 1], hch[:],
                            axis=mybir.AxisListType.X, op=mybir.AluOpType.max,
                            apply_absolute_value=True)
                        nc.scalar.dma_start(
                            h_sp[it][:].rearrange("(n p) d -> p n d", p=128)
                            [:, tsub, ef], hch[:])

                wo_prep_step(it)

            # prefetch first two hT tiles (independent of s_h / the allreduce)
            hT = {}

            def transp_c(it):
                # reuses the (now dead) qxT slots for the transposed-h tiles
                t = wp.tile([128, NE, T], F16, tag="qxT", bufs=2, name="hT16")
                for ib in range(NE):
                    nc.sync.dma_start_transpose(t[:, ib, :],
                                                h_sp[it][:, ib * 128:(ib + 1) * 128])
                hT[it] = t

            transp_c(0)
            transp_c(1)

            # ---- global max|h| -> s_h, output scales (PE-path gate) ----
            nc.vector.tensor_reduce(hmax_red[:], hmax_buf[:], axis=mybir.AxisListType.X,
                                    op=mybir.AluOpType.max)
            part_reduce_pe(hmax_red, sc["red1"], "max")
            allreduce_max(sc["red1"], sc["gmax_h"])
            nc.vector.tensor_scalar(sc["s_h"][0:1, 0:1], sc["gmax_h"][0:1, 0:1],
                                    1.0 / 127.0, None, op0=mybir.AluOpType.mult)
            nc.vector.reciprocal(sc["inv_sh"][0:1, 0:1], sc["s_h"][0:1, 0:1])
            bcast_pe(sc["inv_sh"], bc["inv_sh"])
            nc.vector.tensor_tensor(sc["sco"][0:1, 0:1], sc["ws_o"][0:1, 0:1],
                                    sc["s_h"][0:1, 0:1], op=mybir.AluOpType.mult)
            bcast_pe(sc["sco"], bc["sco"])
            if with_bias:
                nc.vector.reciprocal(sc["inv_sco"][0:1, 0:1], sc["sco"][0:1, 0:1])
                nc.vector.tensor_scalar(bo_sc[0:1, :], bo_sb[0:1, :],
                                        sc["inv_sco"][0:1, 0:1], None,
                                        op0=mybir.AluOpType.mult)

            # ======================= PHASE C =======================
            def quant_c(it):
                # per-ib-block quantize: the first output matmul (which reads
                # ib=0 first) can start right after the first chunk-pair
                t = hT[it]
                t1 = wp.tile([128, TS, E], F32, tag="x_nat", name="t1c")
                t1f = t1.rearrange("p a b -> p (a b)")
                for ib in range(NE):
                    sl = slice(ib * T, (ib + 1) * T)
                    nc.vector.tensor_scalar(
                        t1f[:, sl], t[:, ib, :], bc["inv_sh"][:, 0:1],
                        MAGIC, op0=mybir.AluOpType.mult, op1=mybir.AluOpType.add)
                    nc.vector.tensor_scalar(
                        t[:, ib, :], t1f[:, sl], MAGIC, None,
                        op0=mybir.AluOpType.subtract)
                return t

            quant_c(0)
            quant_c(1)
            for it in range(NT):
                qhT = hT[it]
                del hT[it]
                if it + 2 < NT:
                    transp_c(it + 2)
                    quant_c(it + 2)
                for tsub in range(TS):
                    tcol = slice(tsub * 128, (tsub + 1) * 128)
                    for oh in range(NOH):
                        of = slice(oh * 512, (oh + 1) * 512)
                        ops = psp.tile([128, 512], F32,
                                       tag=("ps_a" if oh % 2 == 0 else "ps_b"),
                                       name="o_ps")
                        for ib in range(NE):
                            nc.tensor.matmul(ops[:], qhT[:, ib, tcol],
                                             qwoT[:, ib, of],
                                             start=(ib == 0),
                                             stop=(not with_bias and ib == NE - 1))
                        if with_bias:
                            nc.tensor.matmul(ops[:], ones_row[0:1, :],
                                             bo_sc[0:1, of],
                                             start=False, stop=True)
                        osb = wp.tile([128, 512], F32, tag="h16", bufs=3, name="osb")
                        nc.scalar.activation(
                            osb[:], ops[:], mybir.ActivationFunctionType.Copy,
                            bias=0.0, scale=bc["sco"][:, 0:1])
                        nc.gpsimd.dma_start(
                            out_ext[it * T:(it + 1) * T, :]
                            .rearrange("(n p) d -> p n d", p=128)
                            [:, tsub, of], osb[:])

    nc.compile()
    return nc


# ----------------------------------------------------------------------------
_CACHE = {}


def _get_nc(key="full", **kw):
    if key not in _CACHE:
        _CACHE[key] = build_nc(**kw)
    return _CACHE[key]


def _make_in_maps(x, memory_keys, memory_values, Wk, bk, Wo, bo):
    x = np.ascontiguousarray(x, dtype=np.float32)
    shared = {
        "memory_keys": np.ascontiguousarray(memory_keys, dtype=np.float32),
        "memory_values": np.ascontiguousarray(memory_values, dtype=np.float32),
        "Wk": np.ascontiguousarray(Wk, dtype=np.float32),
        "bk": np.ascontiguousarray(bk, dtype=np.float32),
        "Wo": np.ascontiguousarray(Wo, dtype=np.float32),
        "bo": np.ascontiguousarray(bo, dtype=np.float32),
    }
    return [dict(shared, x=x[i]) for i in range(x.shape[0])]


def kernel(x, memory_keys, memory_values, Wk, bk, Wv=None, bv=None, Wo=None, bo=None):
    wb = bool(np.any(np.asarray(bo)))
    wbk = bool(np.any(np.asarray(bk)))
    nc = _get_nc(("full", wb, wbk), with_bias=wb, with_bk=wbk)
    in_maps = _make_in_maps(x, memory_keys, memory_values, Wk, bk, Wo, bo)
    res = run_bass_kernel_spmd(nc, in_maps, core_ids=list(range(N_CORES)))
    out = np.stack([res.results[i]["out"] for i in range(N_CORES)], axis=0)
    return out.astype(np.float32)


# ------------------------- benchmarking helper ------------------------------
def bench(inputs, iters=5, nc=None):
    """Time on-device execution with device-resident inputs."""
    import jax
    from jax.sharding import Mesh, PartitionSpec, NamedSharding
    from jax.experimental.shard_map import shard_map
    from concourse import bass2jax as b2j

    if nc is None:
        wb = bool(np.any(np.asarray(inputs["bo"])))
        wbk = bool(np.any(np.asarray(inputs["bk"])))
        nc = _get_nc(("full", wb, wbk), with_bias=wb, with_bk=wbk)
    in_maps = _make_in_maps(inputs["x"], inputs["memory_keys"],
                            inputs["memory_values"], inputs["Wk"], inputs["bk"],
                            inputs["Wo"], inputs["bo"])
    b2j.install_neuronx_cc_hook()

    partition_name = nc.partition_id_tensor.name if nc.partition_id_tensor else None
    in_names, out_names, out_avals, zero_outs = [], [], [], []
    for alloc in nc.m.functions[0].allocations:
        if not isinstance(alloc, mybir.MemoryLocationSet):
            continue
        name = alloc.memorylocations[0].name
        if alloc.kind == "ExternalInput":
            if name != partition_name:
                in_names.append(name)
        elif alloc.kind == "ExternalOutput":
            out_names.append(name)
            shape = tuple(alloc.tensor_shape)
            dtype = mybir.dt.np(alloc.dtype)
            out_avals.append(jax.core.ShapedArray(shape, dtype))
            zero_outs.append(np.zeros(shape, dtype))
    n_params = len(in_names)
    n_outs = len(out_avals)
    in_names = in_names + out_names
    if partition_name is not None:
        in_names.append(partition_name)

    def _body(*args):
        operands = list(args)
        if partition_name is not None:
            operands.append(b2j.partition_id_tensor())
        outs = b2j._bass_exec_p.bind(
            *operands, out_avals=tuple(out_avals), in_names=tuple(in_names),
            out_names=tuple(out_names), lowering_input_output_aliases=(),
            sim_require_finite=True, sim_require_nnan=True, nc=nc)
        return tuple(outs)

    n_cores = len(in_maps)
    devices = jax.devices()[:n_cores]
    mesh = Mesh(np.asarray(devices), ("core",))
    in_specs = (PartitionSpec("core"),) * (n_params + n_outs)
    out_specs = (PartitionSpec("core"),) * len(out_names)
    donate = tuple(range(n_params, n_params + n_outs))
    sharded = jax.jit(
        shard_map(_body, mesh=mesh, in_specs=in_specs, out_specs=out_specs,
                  check_rep=False),
        donate_argnums=donate, keep_unused=True)

    per_core = [[np.asarray(m[nm]) for nm in in_names[:n_params]] for m in in_maps]
    concat_in = [np.concatenate([per_core[c][i] for c in range(n_cores)], axis=0)
                 for i in range(n_params)]
    sh = NamedSharding(mesh, PartitionSpec("core"))
    dev_in = [jax.device_put(a, sh) for a in concat_in]
    for a in dev_in:
        a.block_until_ready()

    times = []
    out_arrs = None
    for i in range(iters + 1):
        dev_zeros = [jax.device_put(
            np.zeros((n_cores * z.shape[0], *z.shape[1:]), z.dtype), sh)
            for z in zero_outs]
        for a in dev_zeros:
            a.block_until_ready()
        t0 = time.perf_counter()
        out_arrs = sharded(*dev_in, *dev_zeros)
        for o in out_arrs:
            o.block_until_ready()
        t1 = time.perf_counter()
        if i > 0:
            times.append(t1 - t0)
    oi = out_names.index("out")
    oshape = out_avals[oi].shape
    out = np.asarray(out_arrs[oi]).reshape(n_cores, *oshape)
    return times, out
